# revision 2
# baseline (speedup 1.0000x reference)
"""Trainium2 Bass kernel for nn_AttentionLayer (masked-diagonal attention), v3.

Per (bs, sq) group of n=64 tokens:
  x2 = layernorm(x) (ddof=1); q = x2 Wq^T + bq; k = x2 Wk^T + bk
  per head h: S_h = q_h k_h^T / 8, mask, softmax rows, take diagonal, sum heads.

Distribution: data-parallel over the 512 (bs, sq) groups across 8 NeuronCores
(64 groups/core), processed as 32 "pairs" (2 groups = 128 rows) in 8
superblocks of 4 pairs.

v3 changes vs the 315us baseline (same PE structure: bf16 projections with
stationary weights + 64x64 per-head score matmuls in diagonal PE quadrants):
 - Mask folded into the LN inv-std scale: masked tokens' x rows are zeroed
   in the (transposed) moving data, so masked score rows/columns are exactly
   0 and exp gives exactly 1; the softmax row-sum Z is corrected by
   subtracting the per-group masked count (one tiny DVE add per pair).
   This removes the mask multiply (DVE) and the mrep upload entirely.
 - Diagonal-extraction identity multiply moved to the pool engine, and the
   identity constant shrunk to [128, 64] (broadcast over heads).
 - Z and diag share one tile and one grouped reduce, reading exp output
   directly (no separate mask product tile).
 - Per-superblock DMA batching: 1 DMA each for x-rowmajor / x-transposed /
   mask-consts / outputs per superblock (vs per-pair) - the baseline spent
   >100us of engine time just issuing DMA descriptors.
 - LN finish batched per superblock: one Newton rsqrt over [128, 4], one
   128x128 block-transpose, 4 row gathers, one ones-matmul broadcast.
"""

import sys

sys.path.insert(0, "/opt/trn_rl_repo")

import numpy as np
import ml_dtypes

import concourse.bass as bass
import concourse.bacc as bacc
import concourse.mybir as mybir
from concourse import tile
from concourse.bass_utils import run_bass_kernel_spmd

F32 = mybir.dt.float32
F32R = mybir.dt.float32r
BF16 = mybir.dt.bfloat16
I32 = mybir.dt.int32
AF = mybir.ActivationFunctionType
ALU = mybir.AluOpType
AXX = mybir.AxisListType.X

DIM = 1024
HEADS = 16
D_K = 64
N_TOK = 64
EPS = 1e-6
N_CORES = 8
N_GROUPS = 512
GROUPS_PER_CORE = N_GROUPS // N_CORES      # 64
PAIRS_PER_CORE = GROUPS_PER_CORE // 2      # 32
SB_PAIRS = 4


def build_graph(n_pairs=PAIRS_PER_CORE, sb_pairs=SB_PAIRS, has_bias=False):
    assert n_pairs % sb_pairs == 0
    n_sb = n_pairs // sb_pairs
    rows_sb = 128 * sb_pairs
    n_half = rows_sb // 512

    nc = bacc.Bacc(None, target_bir_lowering=False)

    xbf_d = nc.declare_dram_parameter(
        "xbf", [n_sb, 128, sb_pairs * DIM], BF16, isOutput=False)
    xt_d = nc.declare_dram_parameter(
        "xt", [n_sb, 128, sb_pairs * DIM], BF16, isOutput=False)
    wq_d = nc.declare_dram_parameter("wqt", [8, 128, DIM], BF16, isOutput=False)
    wk_d = nc.declare_dram_parameter("wkt", [8, 128, DIM], BF16, isOutput=False)
    bqk_d = nc.declare_dram_parameter("bqk", [128, 16], F32, isOutput=False)
    idp_d = nc.declare_dram_parameter("idp", [128, 64], BF16, isOutput=False)
    mn_d = nc.declare_dram_parameter(
        "mn", [n_sb, 128, 3 * sb_pairs], F32, isOutput=False)
    out_d = nc.declare_dram_parameter(
        "out", [n_sb, 128, 2 * sb_pairs], F32, isOutput=True)

    with tile.TileContext(nc) as tc:
        with (
            tc.tile_pool(name="const", bufs=1) as constp,
            tc.tile_pool(name="xbf", bufs=2) as xbfp,
            tc.tile_pool(name="xt", bufs=2) as xtp,
            tc.tile_pool(name="qkt", bufs=3) as qktp,
            tc.tile_pool(name="mn", bufs=3) as mnp,
            tc.tile_pool(name="stats", bufs=4) as statp,
            tc.tile_pool(name="inv", bufs=2) as invp,
            tc.tile_pool(name="psd", bufs=5) as psdp,
            tc.tile_pool(name="zd", bufs=6) as zdp,
            tc.tile_pool(name="res", bufs=2) as resp,
            tc.tile_pool(name="mmps", bufs=4, space=bass.MemorySpace.PSUM) as mmpsp,
            tc.tile_pool(name="scps", bufs=3, space=bass.MemorySpace.PSUM) as scpsp,
            tc.tile_pool(name="ivps", bufs=1, space=bass.MemorySpace.PSUM) as ivpsp,
        ):
            wq_t = [constp.tile([128, DIM], BF16, tag=f"wq{kt}", name=f"wq{kt}")
                    for kt in range(8)]
            wk_t = [constp.tile([128, DIM], BF16, tag=f"wk{kt}", name=f"wk{kt}")
                    for kt in range(8)]
            bqk_sb = constp.tile([128, 16], F32, tag="bqk")
            idp_sb = constp.tile([128, 64], BF16, tag="idp", name="idp")
            invpad = constp.tile([128, 128], F32, tag="invpad", name="invpad")
            onesf = constp.tile([1, 128], F32, tag="onesf", name="onesf")

            def emit_consts():
                nc.gpsimd.memset(invpad[:], 0.0)
                nc.gpsimd.memset(onesf[:], 1.0)
                nc.gpsimd.dma_start(idp_sb[:], idp_d[:])
                if has_bias:
                    nc.gpsimd.dma_start(bqk_sb[:], bqk_d[:])

            xbf_tiles, xt_tiles, mn_tiles = {}, {}, {}
            mv_state, res_tiles = {}, {}

            def emit_x_loads(sb):
                xbf = xbfp.tile([128, sb_pairs * DIM], BF16, tag="xbf",
                                name="xbf")
                for tl in range(sb_pairs):
                    # per-pair slices so bn_stats of pair 0 starts early
                    nc.sync.dma_start(
                        xbf[:, tl * DIM:(tl + 1) * DIM],
                        xbf_d[sb][:, tl * DIM:(tl + 1) * DIM])
                xbf_tiles[sb] = xbf
                xt = xtp.tile([128, sb_pairs * DIM], BF16, tag="xt", name="xt")
                nc.sync.dma_start(xt[:], xt_d[sb])
                xt_tiles[sb] = xt

            def emit_mn_load(sb):
                mn = mnp.tile([128, 3 * sb_pairs], F32, tag="mn", name="mn")
                nc.gpsimd.dma_start(mn[:], mn_d[sb])
                mn_tiles[sb] = mn

            def emit_ln_stats(sb, tl):
                if tl == 0:
                    mv_state[sb] = statp.tile([128, 2 * sb_pairs], F32,
                                              tag="mvsb", name="mvsb")
                mv = mv_state[sb]
                xin = xbf_tiles[sb]
                bno = statp.tile([128, 12], F32, tag="bno")
                nc.vector.bn_stats(bno[:, 0:6],
                                   xin[:, tl * DIM:tl * DIM + 512])
                nc.vector.bn_stats(bno[:, 6:12],
                                   xin[:, tl * DIM + 512:(tl + 1) * DIM])
                mv_ap = mv[:].rearrange("p (a b) -> p a b", a=2)[:, :, tl]
                nc.vector.bn_aggr(mv_ap, bno[:])

            def emit_ln_finish(sb, scale_xt=True):
                # inv-std = rsqrt(var * n/(n-1)) via bit-trick + 2 Newton
                # iters, batched over the superblock's 4 pairs; masked tokens
                # get inv=0 so their (transposed) x rows zero out.
                mv = mv_state.pop(sb)
                var4 = mv[:, sb_pairs:2 * sb_pairs]
                C = 1597463007  # 0x5f3759df
                y0i = statp.tile([128, 4], I32, tag="y0i", name="y0i")
                nc.vector.tensor_scalar(
                    y0i[:], var4.bitcast(I32), 1, None,
                    op0=ALU.arith_shift_right)
                nc.vector.tensor_scalar(
                    y0i[:], y0i[:], -1, C, op0=ALU.mult, op1=ALU.add)
                y = y0i[:].bitcast(F32)
                vh = statp.tile([128, 4], F32, tag="vh", name="vh")
                nc.vector.tensor_scalar_mul(
                    vh[:], var4, -0.5 * float(DIM) / (DIM - 1))
                tt = statp.tile([128, 4], F32, tag="tt", name="tt")
                iv = invpad[:].rearrange("p (a b) -> p a b", b=32)[:, :, 0]
                for it in range(2):
                    nc.vector.tensor_mul(tt[:], y, y)
                    nc.vector.scalar_tensor_tensor(
                        tt[:], tt[:], 1.5, vh[:], op0=ALU.bypass, op1=ALU.mult)
                    nc.vector.tensor_scalar_add(tt[:], tt[:], 1.5)
                    if it == 0:
                        nc.vector.tensor_mul(y, y, tt[:])
                    else:
                        nc.vector.tensor_mul(iv, y, tt[:])
                nc.vector.tensor_mul(iv, iv, mn_tiles[sb][:, 0:sb_pairs])
                # move inv across partitions: one 32x32 block transpose +
                # 4 row gathers + a K=1 ones-matmul broadcast
                invT = invp.tile([128, 128], F32, tag="invT", name="invT")
                nc.vector.transpose(invT[:], invpad[:])
                invrow = invp.tile([1, 512], F32, tag="invrow", name="invrow")
                iT = invT[:].rearrange("(a b) (t c) -> a b t c", b=32, c=32)
                for tl in range(sb_pairs):
                    nc.gpsimd.dma_start(
                        invrow[0:1, tl * 128:(tl + 1) * 128], iT[:, 0, tl, :])
                ibps = ivpsp.tile([128, 512], F32, tag="ibps", name="ibps")
                nc.tensor.matmul(
                    ibps[:], onesf[:].bitcast(F32R), invrow[:].bitcast(F32R),
                    start=True, stop=True)
                invb = invp.tile([128, 512], F32, tag="invb", name="invb")
                nc.scalar.activation(invb[:], ibps[:], AF.Identity,
                                     bias=0.0, scale=1.0)
                if not scale_xt:
                    return invb
                # scale the transposed activations by (mask * 1/std)
                xt = xt_tiles[sb]
                for tl in range(sb_pairs):
                    xtp_ap = xt[:, tl * DIM:(tl + 1) * DIM].rearrange(
                        "p (c r) -> p c r", c=8)
                    invb_b = invb[:, tl * 128:(tl + 1) * 128].unsqueeze(
                        1).broadcast_to((128, 8, 128))
                    nc.vector.tensor_mul(xtp_ap, xtp_ap, invb_b)
                return invb

            def emit_proj_mt(sb, mt, qk_sb, invb0=None):
                xt4 = xt_tiles[sb][:].rearrange(
                    "p (t c r) -> p c t r", t=sb_pairs, c=8)
                for pj, w_t in enumerate((wq_t, wk_t)):
                    ps = mmpsp.tile([128, rows_sb], F32, tag="mmps")
                    for half in range(n_half):
                        for kt in range(8):
                            nc.tensor.matmul(
                                ps[:, half * 512:(half + 1) * 512],
                                w_t[kt][:, mt * 128:(mt + 1) * 128],
                                xt4[:, kt, half * 4:(half + 1) * 4, :],
                                start=(kt == 0),
                                stop=(kt == 7),
                            )
                    dsl = qk_sb[:, mt * 2 * rows_sb + pj * rows_sb:
                                mt * 2 * rows_sb + (pj + 1) * rows_sb]
                    if invb0 is not None:
                        # sb0: apply (mask * 1/std) at evacuation so the PE
                        # needn't wait for the xt-scale chain at kernel start
                        invb_b = invb0[:].unsqueeze(1).broadcast_to(
                            (128, 1, rows_sb))
                        nc.vector.scalar_tensor_tensor(
                            dsl.rearrange("p (a r) -> p a r", a=1),
                            ps[:].rearrange("p (a r) -> p a r", a=1),
                            1.0, invb_b, op0=ALU.mult, op1=ALU.mult)
                        if has_bias:
                            nc.vector.tensor_scalar_add(
                                dsl, dsl,
                                bqk_sb[:, pj * 8 + mt: pj * 8 + mt + 1])
                    elif has_bias:
                        bias_ap = bqk_sb[:, pj * 8 + mt: pj * 8 + mt + 1]
                        if pj == 0:
                            nc.vector.tensor_scalar_add(dsl, ps[:], bias_ap)
                        else:
                            nc.scalar.activation(
                                dsl, ps[:], AF.Identity, bias=bias_ap,
                                scale=1.0)
                    else:
                        nc.scalar.activation(
                            dsl, ps[:], AF.Identity, bias=0.0, scale=1.0)

            def emit_scores_half(sb, tl, half, qk_sb, psd):
                ps = scpsp.tile([128, 512], F32, tag="scps")
                for mtl in range(4):
                    mt = half * 4 + mtl
                    for hp in range(2):
                        for g in range(2):
                            r0 = mt * 2 * rows_sb + tl * 128 + g * 64
                            nc.tensor.matmul(
                                ps[hp * 64:hp * 64 + 64,
                                   mtl * 128 + g * 64: mtl * 128 + g * 64 + 64],
                                qk_sb[hp * 64:hp * 64 + 64, r0:r0 + 64],
                                qk_sb[hp * 64:hp * 64 + 64,
                                      rows_sb + r0:rows_sb + r0 + 64],
                                start=True,
                                stop=True,
                                skip_group_check=True,
                            )
                nc.scalar.activation(
                    psd[:, half * 512:(half + 1) * 512], ps[:], AF.Exp,
                    scale=0.125,
                )

            psd_state = {}

            def emit_scores_open(sb, tl, qk_sb, halves):
                if sb not in res_tiles:
                    res_tiles[sb] = resp.tile([128, 2 * sb_pairs], F32,
                                              tag="res", name="res")
                if (sb, tl) not in psd_state:
                    psd_state[(sb, tl)] = psdp.tile(
                        [128, 2 * DIM], BF16, tag="psd", name="psd")
                psd = psd_state[(sb, tl)]
                for half in halves:
                    emit_scores_half(sb, tl, half, qk_sb, psd)

            def emit_scores_fin(sb, tl):
                mn = mn_tiles[sb]
                res = res_tiles[sb]
                psd = psd_state.pop((sb, tl))
                # diag extraction: exp block * per-head identity (DVE; the
                # pool engine runs this ~5x slower and stalled the pipeline)
                idp_b = idp_sb[:].unsqueeze(1).broadcast_to((128, 16, 64))
                nc.vector.tensor_mul(
                    psd[:, DIM:2 * DIM].rearrange("p (h j) -> p h j", j=64),
                    psd[:, 0:DIM].rearrange("p (h j) -> p h j", j=64),
                    idp_b)
                zd32 = zdp.tile([128, 32], F32, tag="zd32")
                nc.vector.tensor_reduce(
                    zd32[:], psd[:].rearrange("p (b j) -> p b j", j=64),
                    axis=AXX, op=ALU.add,
                )
                # masked k columns contributed exp(0)=1 each; subtract count
                nmneg = mn[:, sb_pairs + 2 * tl:sb_pairs + 2 * tl + 2]
                nc.vector.tensor_add(
                    zd32[:, 0:16].rearrange("p (m g) -> p m g", g=2),
                    zd32[:, 0:16].rearrange("p (m g) -> p m g", g=2),
                    nmneg.unsqueeze(1).broadcast_to((128, 8, 2)))
                rz = zdp.tile([128, 16], F32, tag="rz")
                nc.vector.reciprocal(rz[:], zd32[:, 0:16])
                cb = zdp.tile([128, 16], F32, tag="cb")
                nc.vector.tensor_mul(cb[:], zd32[:, 16:32], rz[:])
                nc.vector.tensor_reduce(
                    res[:, tl * 2:(tl + 1) * 2],
                    cb[:].rearrange("p (m g) -> p g m", g=2),
                    axis=AXX, op=ALU.add,
                )

            def emit_scores_pair(sb, tl, qk_sb):
                emit_scores_open(sb, tl, qk_sb, (0, 1))
                emit_scores_fin(sb, tl)

            def emit_out(sb):
                nc.gpsimd.dma_start(out_d[sb], res_tiles.pop(sb)[:])

            # -------- pipelined driver --------
            emit_x_loads(0)
            emit_consts()
            emit_mn_load(0)
            # spread weight loads over 4 engines' DMA queues - serialized on
            # one queue they gate proj(0) by ~11us
            weng = [nc.sync, nc.scalar, nc.gpsimd]
            for kt in range(8):
                weng[kt % 3].dma_start(wq_t[kt][:], wq_d[kt])
                weng[(kt + 1) % 3].dma_start(wk_t[kt][:], wk_d[kt])
            for tl in range(sb_pairs):
                emit_ln_stats(0, tl)
            invb0 = emit_ln_finish(0, scale_xt=False)
            qk_prev = None
            for sb in range(n_sb):
                if sb + 1 < n_sb:
                    emit_x_loads(sb + 1)
                    emit_mn_load(sb + 1)
                qk_sb = qktp.tile([128, 16 * rows_sb], BF16, tag="qk",
                                  name="qk_sb")
                last = sb == n_sb - 1
                for step in range(8):
                    if sb + 1 < n_sb and step < sb_pairs:
                        emit_ln_stats(sb + 1, step)
                    if qk_prev is not None and step % 2 == 1:
                        emit_scores_pair(sb - 1, step // 2, qk_prev)
                    if sb + 1 < n_sb and step == 4:
                        # all 4 pairs' stats are in by step 3; finishing here
                        # gives the inv/xt-scale chain ~3 proj-mts of slack
                        # before proj(sb+1, 0) consumes the scaled xt
                        emit_ln_finish(sb + 1)
                    emit_proj_mt(sb, step, qk_sb,
                                 invb0=invb0 if sb == 0 else None)
                    if last and step >= 4:
                        # heads 0-7 only need proj mts 0-3: start the final
                        # superblock's half-0 scores early to shrink the tail
                        emit_scores_open(sb, step - 4, qk_sb, (0,))
                if qk_prev is not None:
                    emit_out(sb - 1)
                qk_prev = qk_sb
            for tl in range(sb_pairs):
                emit_scores_open(n_sb - 1, tl, qk_prev, (1,))
                emit_scores_fin(n_sb - 1, tl)
            emit_out(n_sb - 1)

    nc.compile()
    return nc


def prepare_host_inputs(x, mask, alpha, bias, Wq, bq, Wk, bk,
                        n_pairs=PAIRS_PER_CORE, sb_pairs=SB_PAIRS,
                        n_cores=N_CORES):
    """Fold LN affine params + mean-centering into weights, shard, build
    per-core in_maps.  Host work is data formatting only (reshape/transpose/
    dtype-cast) plus weight preprocessing."""
    x = np.asarray(x, np.float32)
    mask = np.asarray(mask, bool)
    alpha = np.asarray(alpha, np.float64)
    bias = np.asarray(bias, np.float64)
    Wq = np.asarray(Wq, np.float64)
    Wk = np.asarray(Wk, np.float64)
    bq = np.asarray(bq, np.float64)
    bk = np.asarray(bk, np.float64)
    n_sb = n_pairs // sb_pairs

    # q = alpha*(x-mean)/std @ Wq.T + (bias @ Wq.T + bq)
    #   = (x-mean)/std @ Wq'.T + bq'   with Wq' = Wq*alpha
    # and since sum_k (x-mean) = 0, Wq' can be row-centered exactly:
    Wqp = Wq * alpha[None, :]
    Wkp = Wk * alpha[None, :]
    Wqc = Wqp - Wqp.mean(axis=1, keepdims=True)
    Wkc = Wkp - Wkp.mean(axis=1, keepdims=True)
    bqp = (bq + Wq @ bias).astype(np.float32)
    bkp = (bk + Wk @ bias).astype(np.float32)

    wqt = np.ascontiguousarray(
        Wqc.T.reshape(8, 128, DIM).astype(ml_dtypes.bfloat16))
    wkt = np.ascontiguousarray(
        Wkc.T.reshape(8, 128, DIM).astype(ml_dtypes.bfloat16))
    bqk = np.ascontiguousarray(
        np.stack([bqp.reshape(8, 128), bkp.reshape(8, 128)]).reshape(16, 128).T)

    idp = np.zeros((128, 64), ml_dtypes.bfloat16)
    p = np.arange(128) % 64
    j = np.arange(64)
    idp[p[:, None] == j[None, :]] = 1.0

    n_groups = x.size // (N_TOK * DIM)
    xg = x.reshape(n_groups, N_TOK, DIM)
    mg = mask.reshape(n_groups, N_TOK)
    gpc = 2 * n_pairs
    in_maps = []
    for c in range(n_cores):
        xs = xg[c * gpc:(c + 1) * gpc].reshape(n_sb, sb_pairs, 128, DIM)
        xsb = xs.astype(ml_dtypes.bfloat16)
        # xbf[sb, p, tl*1024 + d] = x[sb, tl, p, d]
        xbf = np.ascontiguousarray(
            xsb.transpose(0, 2, 1, 3).reshape(n_sb, 128, sb_pairs * DIM))
        # xt[sb, p, tl*1024 + c*128 + r] = x[sb, tl, r, c*128+p]
        xt = np.ascontiguousarray(
            xsb.reshape(n_sb, sb_pairs, 128, 8, 128).transpose(0, 4, 1, 3, 2)
            .reshape(n_sb, 128, sb_pairs * DIM))
        # mn[sb, p, 0:4]: mask of pair-token p (pair tl); [4:12]: -(masked
        # count of group g in pair tl) at col 4 + tl*2 + g (same for all p)
        ms = mg[c * gpc:(c + 1) * gpc].reshape(n_sb, sb_pairs, 2, 64)
        mskp = ms.transpose(0, 2, 3, 1).reshape(n_sb, 128, sb_pairs)
        nm = (64 - ms.sum(-1)).astype(np.float32)      # [n_sb, tl, g]
        negnm = np.broadcast_to(
            -nm.reshape(n_sb, 1, 2 * sb_pairs), (n_sb, 128, 2 * sb_pairs))
        mn = np.ascontiguousarray(np.concatenate(
            [mskp.astype(np.float32), negnm], axis=2))
        in_maps.append({
            "xbf": xbf, "xt": xt, "wqt": wqt, "wkt": wkt, "bqk": bqk,
            "idp": idp, "mn": mn,
        })
    return in_maps


def postprocess(results, mask, n_pairs=PAIRS_PER_CORE, sb_pairs=SB_PAIRS,
                n_cores=N_CORES):
    """Gather per-core results, sum head-parity halves, apply mask fixup."""
    mask = np.asarray(mask, bool)
    n_sb = n_pairs // sb_pairs
    out = np.empty((N_GROUPS, N_TOK), np.float32)
    gpc = 2 * n_pairs
    for c in range(n_cores):
        res = results[c]["out"]                    # [n_sb, 128, 2*sb_pairs]
        r = res.reshape(n_sb, 2, 64, sb_pairs, 2)  # [sb, hp, i, tl, g]
        summed = r[:, 0] + r[:, 1]                 # [sb, i, tl, g]
        out[c * gpc:(c + 1) * gpc] = summed.transpose(0, 2, 3, 1).reshape(
            gpc, N_TOK)
    out = out.reshape(mask.shape)
    out[~mask] = 0.25
    return out


_NC_CACHE = {}


def _get_graph(has_bias):
    key = ("nc", has_bias)
    if key not in _NC_CACHE:
        _NC_CACHE[key] = build_graph(has_bias=has_bias)
    return _NC_CACHE[key]


def kernel(x, mask, alpha, bias, Wq, bq, Wk, bk, _trace=False,
           _trace_kwargs=None):
    bqp = np.asarray(bq, np.float64) + np.asarray(Wq, np.float64) @ np.asarray(bias, np.float64)
    bkp = np.asarray(bk, np.float64) + np.asarray(Wk, np.float64) @ np.asarray(bias, np.float64)
    has_bias = bool(np.any(bqp != 0) or np.any(bkp != 0))
    nc = _get_graph(has_bias)
    in_maps = prepare_host_inputs(x, mask, alpha, bias, Wq, bq, Wk, bk)
    kw = {}
    if _trace:
        kw = dict(trace=True, **(_trace_kwargs or {}))
    r = run_bass_kernel_spmd(nc, in_maps, core_ids=list(range(N_CORES)), **kw)
    out = postprocess(r.results, mask)
    if _trace:
        kernel.last_exec_time_ns = r.exec_time_ns
        kernel.last_results = r
    return out


# revision 3
# speedup vs baseline: 1.0031x; 1.0031x over previous
"""Trainium2 Bass kernel for nn_AttentionLayer (masked-diagonal attention), v3.

Per (bs, sq) group of n=64 tokens:
  x2 = layernorm(x) (ddof=1); q = x2 Wq^T + bq; k = x2 Wk^T + bk
  per head h: S_h = q_h k_h^T / 8, mask, softmax rows, take diagonal, sum heads.

Distribution: data-parallel over the 512 (bs, sq) groups across 8 NeuronCores
(64 groups/core), processed as 32 "pairs" (2 groups = 128 rows) in 8
superblocks of 4 pairs.

v3 changes vs the 315us baseline (same PE structure: bf16 projections with
stationary weights + 64x64 per-head score matmuls in diagonal PE quadrants):
 - Mask folded into the LN inv-std scale: masked tokens' x rows are zeroed
   in the (transposed) moving data, so masked score rows/columns are exactly
   0 and exp gives exactly 1; the softmax row-sum Z is corrected by
   subtracting the per-group masked count (one tiny DVE add per pair).
   This removes the mask multiply (DVE) and the mrep upload entirely.
 - Diagonal-extraction identity multiply moved to the pool engine, and the
   identity constant shrunk to [128, 64] (broadcast over heads).
 - Z and diag share one tile and one grouped reduce, reading exp output
   directly (no separate mask product tile).
 - Per-superblock DMA batching: 1 DMA each for x-rowmajor / x-transposed /
   mask-consts / outputs per superblock (vs per-pair) - the baseline spent
   >100us of engine time just issuing DMA descriptors.
 - LN finish batched per superblock: one Newton rsqrt over [128, 4], one
   128x128 block-transpose, 4 row gathers, one ones-matmul broadcast.
"""

import sys

sys.path.insert(0, "/opt/trn_rl_repo")

import numpy as np
import ml_dtypes

import concourse.bass as bass
import concourse.bacc as bacc
import concourse.mybir as mybir
from concourse import tile
from concourse.bass_utils import run_bass_kernel_spmd

F32 = mybir.dt.float32
F32R = mybir.dt.float32r
BF16 = mybir.dt.bfloat16
I32 = mybir.dt.int32
AF = mybir.ActivationFunctionType
ALU = mybir.AluOpType
AXX = mybir.AxisListType.X

DIM = 1024
HEADS = 16
D_K = 64
N_TOK = 64
EPS = 1e-6
N_CORES = 8
N_GROUPS = 512
GROUPS_PER_CORE = N_GROUPS // N_CORES      # 64
PAIRS_PER_CORE = GROUPS_PER_CORE // 2      # 32
SB_PAIRS = 4


def build_graph(n_pairs=PAIRS_PER_CORE, sb_pairs=SB_PAIRS, has_bias=False):
    assert n_pairs % sb_pairs == 0
    n_sb = n_pairs // sb_pairs
    rows_sb = 128 * sb_pairs
    n_half = rows_sb // 512

    nc = bacc.Bacc(None, target_bir_lowering=False)

    xbf_d = nc.declare_dram_parameter(
        "xbf", [n_sb, 128, sb_pairs * DIM], BF16, isOutput=False)
    xt_d = nc.declare_dram_parameter(
        "xt", [n_sb, 128, sb_pairs * DIM], BF16, isOutput=False)
    wq_d = nc.declare_dram_parameter("wqt", [8, 128, DIM], BF16, isOutput=False)
    wk_d = nc.declare_dram_parameter("wkt", [8, 128, DIM], BF16, isOutput=False)
    bqk_d = nc.declare_dram_parameter("bqk", [128, 16], F32, isOutput=False)
    idp_d = nc.declare_dram_parameter("idp", [128, 64], BF16, isOutput=False)
    mn_d = nc.declare_dram_parameter(
        "mn", [n_sb, 128, 3 * sb_pairs], F32, isOutput=False)
    out_d = nc.declare_dram_parameter(
        "out", [n_sb, 128, 2 * sb_pairs], F32, isOutput=True)

    with tile.TileContext(nc) as tc:
        with (
            tc.tile_pool(name="const", bufs=1) as constp,
            tc.tile_pool(name="xbf", bufs=2) as xbfp,
            tc.tile_pool(name="xt", bufs=2) as xtp,
            tc.tile_pool(name="qkt", bufs=3) as qktp,
            tc.tile_pool(name="mn", bufs=3) as mnp,
            tc.tile_pool(name="stats", bufs=4) as statp,
            tc.tile_pool(name="inv", bufs=2) as invp,
            tc.tile_pool(name="psd", bufs=5) as psdp,
            tc.tile_pool(name="zd", bufs=6) as zdp,
            tc.tile_pool(name="res", bufs=2) as resp,
            tc.tile_pool(name="mmps", bufs=4, space=bass.MemorySpace.PSUM) as mmpsp,
            tc.tile_pool(name="scps", bufs=3, space=bass.MemorySpace.PSUM) as scpsp,
            tc.tile_pool(name="ivps", bufs=1, space=bass.MemorySpace.PSUM) as ivpsp,
        ):
            wq_t = [constp.tile([128, DIM], BF16, tag=f"wq{kt}", name=f"wq{kt}")
                    for kt in range(8)]
            wk_t = [constp.tile([128, DIM], BF16, tag=f"wk{kt}", name=f"wk{kt}")
                    for kt in range(8)]
            bqk_sb = constp.tile([128, 16], F32, tag="bqk")
            idp_sb = constp.tile([128, 64], BF16, tag="idp", name="idp")
            invpad = constp.tile([128, 128], F32, tag="invpad", name="invpad")
            onesf = constp.tile([1, 128], F32, tag="onesf", name="onesf")

            def emit_consts():
                nc.gpsimd.memset(invpad[:], 0.0)
                nc.gpsimd.memset(onesf[:], 1.0)
                nc.gpsimd.dma_start(idp_sb[:], idp_d[:])
                if has_bias:
                    nc.gpsimd.dma_start(bqk_sb[:], bqk_d[:])

            xbf_tiles, xt_tiles, mn_tiles = {}, {}, {}
            mv_state, res_tiles = {}, {}

            def emit_x_loads(sb):
                xbf = xbfp.tile([128, sb_pairs * DIM], BF16, tag="xbf",
                                name="xbf")
                for tl in range(sb_pairs):
                    # per-pair slices so bn_stats of pair 0 starts early
                    nc.sync.dma_start(
                        xbf[:, tl * DIM:(tl + 1) * DIM],
                        xbf_d[sb][:, tl * DIM:(tl + 1) * DIM])
                xbf_tiles[sb] = xbf
                xt = xtp.tile([128, sb_pairs * DIM], BF16, tag="xt", name="xt")
                nc.sync.dma_start(xt[:], xt_d[sb])
                xt_tiles[sb] = xt

            def emit_mn_load(sb):
                mn = mnp.tile([128, 3 * sb_pairs], F32, tag="mn", name="mn")
                nc.gpsimd.dma_start(mn[:], mn_d[sb])
                mn_tiles[sb] = mn

            def emit_ln_stats(sb, tl):
                if tl == 0:
                    mv_state[sb] = statp.tile([128, 2 * sb_pairs], F32,
                                              tag="mvsb", name="mvsb")
                mv = mv_state[sb]
                xin = xbf_tiles[sb]
                bno = statp.tile([128, 12], F32, tag="bno")
                nc.vector.bn_stats(bno[:, 0:6],
                                   xin[:, tl * DIM:tl * DIM + 512])
                nc.vector.bn_stats(bno[:, 6:12],
                                   xin[:, tl * DIM + 512:(tl + 1) * DIM])
                mv_ap = mv[:].rearrange("p (a b) -> p a b", a=2)[:, :, tl]
                nc.vector.bn_aggr(mv_ap, bno[:])

            def emit_inv_broadcast(invrow):
                ibps = ivpsp.tile([128, 512], F32, tag="ibps", name="ibps")
                nc.tensor.matmul(
                    ibps[:], onesf[:].bitcast(F32R), invrow[:].bitcast(F32R),
                    start=True, stop=True)
                invb = invp.tile([128, 512], F32, tag="invb", name="invb")
                nc.scalar.activation(invb[:], ibps[:], AF.Identity,
                                     bias=0.0, scale=1.0)
                return invb

            def emit_ln_finish(sb, scale_xt=True, broadcast=True):
                # inv-std = rsqrt(var * n/(n-1)) via bit-trick + 2 Newton
                # iters, batched over the superblock's 4 pairs; masked tokens
                # get inv=0 so their (transposed) x rows zero out.
                mv = mv_state.pop(sb)
                var4 = mv[:, sb_pairs:2 * sb_pairs]
                C = 1597463007  # 0x5f3759df
                y0i = statp.tile([128, 4], I32, tag="y0i", name="y0i")
                nc.vector.tensor_scalar(
                    y0i[:], var4.bitcast(I32), 1, None,
                    op0=ALU.arith_shift_right)
                nc.vector.tensor_scalar(
                    y0i[:], y0i[:], -1, C, op0=ALU.mult, op1=ALU.add)
                y = y0i[:].bitcast(F32)
                vh = statp.tile([128, 4], F32, tag="vh", name="vh")
                nc.vector.tensor_scalar_mul(
                    vh[:], var4, -0.5 * float(DIM) / (DIM - 1))
                tt = statp.tile([128, 4], F32, tag="tt", name="tt")
                iv = invpad[:].rearrange("p (a b) -> p a b", b=32)[:, :, 0]
                for it in range(2):
                    nc.vector.tensor_mul(tt[:], y, y)
                    nc.vector.scalar_tensor_tensor(
                        tt[:], tt[:], 1.5, vh[:], op0=ALU.bypass, op1=ALU.mult)
                    nc.vector.tensor_scalar_add(tt[:], tt[:], 1.5)
                    if it == 0:
                        nc.vector.tensor_mul(y, y, tt[:])
                    else:
                        nc.vector.tensor_mul(iv, y, tt[:])
                nc.vector.tensor_mul(iv, iv, mn_tiles[sb][:, 0:sb_pairs])
                # move inv across partitions: one 32x32 block transpose +
                # 4 row gathers + a K=1 ones-matmul broadcast
                invT = invp.tile([128, 128], F32, tag="invT", name="invT")
                nc.vector.transpose(invT[:], invpad[:])
                invrow = invp.tile([1, 512], F32, tag="invrow", name="invrow")
                iT = invT[:].rearrange("(a b) (t c) -> a b t c", b=32, c=32)
                for tl in range(sb_pairs):
                    nc.gpsimd.dma_start(
                        invrow[0:1, tl * 128:(tl + 1) * 128], iT[:, 0, tl, :])
                if not broadcast:
                    # sb0: the ones-matmul is emitted mid-body-0 instead, so
                    # it doesn't head the in-order PE queue gated on this
                    # whole DVE chain
                    return invrow
                invb = emit_inv_broadcast(invrow)
                if not scale_xt:
                    return invb
                # scale the transposed activations by (mask * 1/std)
                xt = xt_tiles[sb]
                for tl in range(sb_pairs):
                    xtp_ap = xt[:, tl * DIM:(tl + 1) * DIM].rearrange(
                        "p (c r) -> p c r", c=8)
                    invb_b = invb[:, tl * 128:(tl + 1) * 128].unsqueeze(
                        1).broadcast_to((128, 8, 128))
                    nc.vector.tensor_mul(xtp_ap, xtp_ap, invb_b)
                return invb

            def emit_proj_mt(sb, mt, qk_sb, invb0=None, defer=None):
                xt4 = xt_tiles[sb][:].rearrange(
                    "p (t c r) -> p c t r", t=sb_pairs, c=8)
                for pj, w_t in enumerate((wq_t, wk_t)):
                    ps = mmpsp.tile([128, rows_sb], F32, tag="mmps")
                    for half in range(n_half):
                        for kt in range(8):
                            nc.tensor.matmul(
                                ps[:, half * 512:(half + 1) * 512],
                                w_t[kt][:, mt * 128:(mt + 1) * 128],
                                xt4[:, kt, half * 4:(half + 1) * 4, :],
                                start=(kt == 0),
                                stop=(kt == 7),
                            )
                    dsl = qk_sb[:, mt * 2 * rows_sb + pj * rows_sb:
                                mt * 2 * rows_sb + (pj + 1) * rows_sb]
                    if defer is not None:
                        defer.append((dsl, ps))
                        continue
                    if invb0 is not None:
                        # sb0: apply (mask * 1/std) at evacuation so the PE
                        # needn't wait for the xt-scale chain at kernel start
                        invb_b = invb0[:].unsqueeze(1).broadcast_to(
                            (128, 1, rows_sb))
                        nc.vector.scalar_tensor_tensor(
                            dsl.rearrange("p (a r) -> p a r", a=1),
                            ps[:].rearrange("p (a r) -> p a r", a=1),
                            1.0, invb_b, op0=ALU.mult, op1=ALU.mult)
                        if has_bias:
                            nc.vector.tensor_scalar_add(
                                dsl, dsl,
                                bqk_sb[:, pj * 8 + mt: pj * 8 + mt + 1])
                    elif has_bias:
                        bias_ap = bqk_sb[:, pj * 8 + mt: pj * 8 + mt + 1]
                        if pj == 0:
                            nc.vector.tensor_scalar_add(dsl, ps[:], bias_ap)
                        else:
                            nc.scalar.activation(
                                dsl, ps[:], AF.Identity, bias=bias_ap,
                                scale=1.0)
                    else:
                        nc.scalar.activation(
                            dsl, ps[:], AF.Identity, bias=0.0, scale=1.0)

            def emit_scores_half(sb, tl, half, qk_sb, psd):
                ps = scpsp.tile([128, 512], F32, tag="scps")
                for mtl in range(4):
                    mt = half * 4 + mtl
                    for hp in range(2):
                        for g in range(2):
                            r0 = mt * 2 * rows_sb + tl * 128 + g * 64
                            nc.tensor.matmul(
                                ps[hp * 64:hp * 64 + 64,
                                   mtl * 128 + g * 64: mtl * 128 + g * 64 + 64],
                                qk_sb[hp * 64:hp * 64 + 64, r0:r0 + 64],
                                qk_sb[hp * 64:hp * 64 + 64,
                                      rows_sb + r0:rows_sb + r0 + 64],
                                start=True,
                                stop=True,
                                skip_group_check=True,
                            )
                nc.scalar.activation(
                    psd[:, half * 512:(half + 1) * 512], ps[:], AF.Exp,
                    scale=0.125,
                )

            psd_state = {}

            def emit_scores_open(sb, tl, qk_sb, halves):
                if sb not in res_tiles:
                    res_tiles[sb] = resp.tile([128, 2 * sb_pairs], F32,
                                              tag="res", name="res")
                if (sb, tl) not in psd_state:
                    psd_state[(sb, tl)] = psdp.tile(
                        [128, 2 * DIM], BF16, tag="psd", name="psd")
                psd = psd_state[(sb, tl)]
                for half in halves:
                    emit_scores_half(sb, tl, half, qk_sb, psd)

            def emit_scores_fin(sb, tl):
                mn = mn_tiles[sb]
                res = res_tiles[sb]
                psd = psd_state.pop((sb, tl))
                # diag extraction: exp block * per-head identity (DVE; the
                # pool engine runs this ~5x slower and stalled the pipeline)
                idp_b = idp_sb[:].unsqueeze(1).broadcast_to((128, 16, 64))
                nc.vector.tensor_mul(
                    psd[:, DIM:2 * DIM].rearrange("p (h j) -> p h j", j=64),
                    psd[:, 0:DIM].rearrange("p (h j) -> p h j", j=64),
                    idp_b)
                zd32 = zdp.tile([128, 32], F32, tag="zd32")
                nc.vector.tensor_reduce(
                    zd32[:], psd[:].rearrange("p (b j) -> p b j", j=64),
                    axis=AXX, op=ALU.add,
                )
                # masked k columns contributed exp(0)=1 each; subtract count
                nmneg = mn[:, sb_pairs + 2 * tl:sb_pairs + 2 * tl + 2]
                nc.vector.tensor_add(
                    zd32[:, 0:16].rearrange("p (m g) -> p m g", g=2),
                    zd32[:, 0:16].rearrange("p (m g) -> p m g", g=2),
                    nmneg.unsqueeze(1).broadcast_to((128, 8, 2)))
                rz = zdp.tile([128, 16], F32, tag="rz")
                nc.vector.reciprocal(rz[:], zd32[:, 0:16])
                cb = zdp.tile([128, 16], F32, tag="cb")
                nc.vector.tensor_mul(cb[:], zd32[:, 16:32], rz[:])
                nc.vector.tensor_reduce(
                    res[:, tl * 2:(tl + 1) * 2],
                    cb[:].rearrange("p (m g) -> p g m", g=2),
                    axis=AXX, op=ALU.add,
                )

            def emit_scores_pair(sb, tl, qk_sb):
                emit_scores_open(sb, tl, qk_sb, (0, 1))
                emit_scores_fin(sb, tl)

            def emit_out(sb):
                nc.gpsimd.dma_start(out_d[sb], res_tiles.pop(sb)[:])

            # -------- pipelined driver --------
            emit_x_loads(0)
            emit_consts()
            emit_mn_load(0)
            # spread weight loads over 4 engines' DMA queues - serialized on
            # one queue they gate proj(0) by ~11us
            weng = [nc.sync, nc.scalar, nc.gpsimd]
            for kt in range(8):
                weng[kt % 3].dma_start(wq_t[kt][:], wq_d[kt])
                weng[(kt + 1) % 3].dma_start(wk_t[kt][:], wk_d[kt])
            for tl in range(sb_pairs):
                emit_ln_stats(0, tl)
            invrow0 = emit_ln_finish(0, scale_xt=False, broadcast=False)
            invb0 = None
            qk_prev = None
            for sb in range(n_sb):
                if sb + 1 < n_sb:
                    emit_x_loads(sb + 1)
                    emit_mn_load(sb + 1)
                qk_sb = qktp.tile([128, 16 * rows_sb], BF16, tag="qk",
                                  name="qk_sb")
                last = sb == n_sb - 1
                deferred = [] if sb == 0 else None
                for step in range(8):
                    if sb + 1 < n_sb and step < sb_pairs:
                        emit_ln_stats(sb + 1, step)
                    if qk_prev is not None and step % 2 == 1:
                        emit_scores_pair(sb - 1, step // 2, qk_prev)
                    if sb + 1 < n_sb and step == 4:
                        # all 4 pairs' stats are in by step 3; finishing here
                        # gives the inv/xt-scale chain ~3 proj-mts of slack
                        # before proj(sb+1, 0) consumes the scaled xt
                        emit_ln_finish(sb + 1)
                    if sb == 0 and step == 2:
                        # PE reaches this ones-matmul after proj mts 0-1,
                        # right as the sb0 inv chain lands; flush the two
                        # deferred evacuations, then evacuate inline
                        invb0 = emit_inv_broadcast(invrow0)
                        for dsl_, ps_ in deferred:
                            invb_b = invb0[:].unsqueeze(1).broadcast_to(
                                (128, 1, rows_sb))
                            nc.vector.scalar_tensor_tensor(
                                dsl_.rearrange("p (a r) -> p a r", a=1),
                                ps_[:].rearrange("p (a r) -> p a r", a=1),
                                1.0, invb_b, op0=ALU.mult, op1=ALU.mult)
                        deferred = None
                    emit_proj_mt(sb, step, qk_sb,
                                 invb0=invb0 if sb == 0 else None,
                                 defer=deferred)
                    if last and step >= 4:
                        # heads 0-7 only need proj mts 0-3: start the final
                        # superblock's half-0 scores early to shrink the tail
                        emit_scores_open(sb, step - 4, qk_sb, (0,))
                if qk_prev is not None:
                    emit_out(sb - 1)
                qk_prev = qk_sb
            for tl in range(sb_pairs):
                emit_scores_open(n_sb - 1, tl, qk_prev, (1,))
            for tl in range(sb_pairs):
                emit_scores_fin(n_sb - 1, tl)
            emit_out(n_sb - 1)

    nc.compile()
    return nc


def prepare_host_inputs(x, mask, alpha, bias, Wq, bq, Wk, bk,
                        n_pairs=PAIRS_PER_CORE, sb_pairs=SB_PAIRS,
                        n_cores=N_CORES):
    """Fold LN affine params + mean-centering into weights, shard, build
    per-core in_maps.  Host work is data formatting only (reshape/transpose/
    dtype-cast) plus weight preprocessing."""
    x = np.asarray(x, np.float32)
    mask = np.asarray(mask, bool)
    alpha = np.asarray(alpha, np.float64)
    bias = np.asarray(bias, np.float64)
    Wq = np.asarray(Wq, np.float64)
    Wk = np.asarray(Wk, np.float64)
    bq = np.asarray(bq, np.float64)
    bk = np.asarray(bk, np.float64)
    n_sb = n_pairs // sb_pairs

    # q = alpha*(x-mean)/std @ Wq.T + (bias @ Wq.T + bq)
    #   = (x-mean)/std @ Wq'.T + bq'   with Wq' = Wq*alpha
    # and since sum_k (x-mean) = 0, Wq' can be row-centered exactly:
    Wqp = Wq * alpha[None, :]
    Wkp = Wk * alpha[None, :]
    Wqc = Wqp - Wqp.mean(axis=1, keepdims=True)
    Wkc = Wkp - Wkp.mean(axis=1, keepdims=True)
    bqp = (bq + Wq @ bias).astype(np.float32)
    bkp = (bk + Wk @ bias).astype(np.float32)

    wqt = np.ascontiguousarray(
        Wqc.T.reshape(8, 128, DIM).astype(ml_dtypes.bfloat16))
    wkt = np.ascontiguousarray(
        Wkc.T.reshape(8, 128, DIM).astype(ml_dtypes.bfloat16))
    bqk = np.ascontiguousarray(
        np.stack([bqp.reshape(8, 128), bkp.reshape(8, 128)]).reshape(16, 128).T)

    idp = np.zeros((128, 64), ml_dtypes.bfloat16)
    p = np.arange(128) % 64
    j = np.arange(64)
    idp[p[:, None] == j[None, :]] = 1.0

    n_groups = x.size // (N_TOK * DIM)
    xg = x.reshape(n_groups, N_TOK, DIM)
    mg = mask.reshape(n_groups, N_TOK)
    gpc = 2 * n_pairs
    in_maps = []
    for c in range(n_cores):
        xs = xg[c * gpc:(c + 1) * gpc].reshape(n_sb, sb_pairs, 128, DIM)
        xsb = xs.astype(ml_dtypes.bfloat16)
        # xbf[sb, p, tl*1024 + d] = x[sb, tl, p, d]
        xbf = np.ascontiguousarray(
            xsb.transpose(0, 2, 1, 3).reshape(n_sb, 128, sb_pairs * DIM))
        # xt[sb, p, tl*1024 + c*128 + r] = x[sb, tl, r, c*128+p]
        xt = np.ascontiguousarray(
            xsb.reshape(n_sb, sb_pairs, 128, 8, 128).transpose(0, 4, 1, 3, 2)
            .reshape(n_sb, 128, sb_pairs * DIM))
        # mn[sb, p, 0:4]: mask of pair-token p (pair tl); [4:12]: -(masked
        # count of group g in pair tl) at col 4 + tl*2 + g (same for all p)
        ms = mg[c * gpc:(c + 1) * gpc].reshape(n_sb, sb_pairs, 2, 64)
        mskp = ms.transpose(0, 2, 3, 1).reshape(n_sb, 128, sb_pairs)
        nm = (64 - ms.sum(-1)).astype(np.float32)      # [n_sb, tl, g]
        negnm = np.broadcast_to(
            -nm.reshape(n_sb, 1, 2 * sb_pairs), (n_sb, 128, 2 * sb_pairs))
        mn = np.ascontiguousarray(np.concatenate(
            [mskp.astype(np.float32), negnm], axis=2))
        in_maps.append({
            "xbf": xbf, "xt": xt, "wqt": wqt, "wkt": wkt, "bqk": bqk,
            "idp": idp, "mn": mn,
        })
    return in_maps


def postprocess(results, mask, n_pairs=PAIRS_PER_CORE, sb_pairs=SB_PAIRS,
                n_cores=N_CORES):
    """Gather per-core results, sum head-parity halves, apply mask fixup."""
    mask = np.asarray(mask, bool)
    n_sb = n_pairs // sb_pairs
    out = np.empty((N_GROUPS, N_TOK), np.float32)
    gpc = 2 * n_pairs
    for c in range(n_cores):
        res = results[c]["out"]                    # [n_sb, 128, 2*sb_pairs]
        r = res.reshape(n_sb, 2, 64, sb_pairs, 2)  # [sb, hp, i, tl, g]
        summed = r[:, 0] + r[:, 1]                 # [sb, i, tl, g]
        out[c * gpc:(c + 1) * gpc] = summed.transpose(0, 2, 3, 1).reshape(
            gpc, N_TOK)
    out = out.reshape(mask.shape)
    out[~mask] = 0.25
    return out


_NC_CACHE = {}


def _get_graph(has_bias):
    key = ("nc", has_bias)
    if key not in _NC_CACHE:
        _NC_CACHE[key] = build_graph(has_bias=has_bias)
    return _NC_CACHE[key]


def kernel(x, mask, alpha, bias, Wq, bq, Wk, bk, _trace=False,
           _trace_kwargs=None):
    bqp = np.asarray(bq, np.float64) + np.asarray(Wq, np.float64) @ np.asarray(bias, np.float64)
    bkp = np.asarray(bk, np.float64) + np.asarray(Wk, np.float64) @ np.asarray(bias, np.float64)
    has_bias = bool(np.any(bqp != 0) or np.any(bkp != 0))
    nc = _get_graph(has_bias)
    in_maps = prepare_host_inputs(x, mask, alpha, bias, Wq, bq, Wk, bk)
    kw = {}
    if _trace:
        kw = dict(trace=True, **(_trace_kwargs or {}))
    r = run_bass_kernel_spmd(nc, in_maps, core_ids=list(range(N_CORES)), **kw)
    out = postprocess(r.results, mask)
    if _trace:
        kernel.last_exec_time_ns = r.exec_time_ns
        kernel.last_results = r
    return out
